# revision 58
# baseline (speedup 1.0000x reference)
"""Trainium2 Bass kernel for nn_DynamicBlock (sparse-token attention + MLP block).

Contract: kernel(**inputs) takes the FULL unsharded inputs (as produced by
reference.setup_inputs()) and returns the FULL [B, T, D] output.

Sharding: 8 cores = 4 batches x 2 interleaved query-halves. Each core:
 - computes rmsnorm + K/V projections (+rope on K) for its batch over all T,
 - processes its 256 selected queries: Q proj + rope, causal attention over
   all T keys (GQA 16 q-heads / 8 kv-heads), o-proj, MLP, gated update,
 - returns the 256 updated rows; the host scatters them into a copy of
   hidden_states.

v2 layout/scheduling notes (vs the original baseline, 453us -> ~296us):
 - All big DRAM tensors are partition-major [128, ...] and laid out exactly
   like their SBUF tiles, so every load is ONE DMA with >=2KB-contiguous
   per-partition descriptors (the 1KB-descriptor loads were ~2x slower).
 - Small constants are packed into three tensors (mask consts first and
   tiny, so the scheduler-hoisted mask ops can't stall the vector queue
   behind a fat const DMA during the startup DMA ramp).
 - rmsnorm sums are broadcast to all 128 partitions by the ones-matmul
   itself (lhsT = ones [128,128]); 1/sqrt via Ln->Exp(-0.5) on the scalar
   engine (wide), removing the slow 1-lane DVE reciprocals entirely.
 - Softmax denominators: rowsums accumulate in the ctx PSUM (extra ones col
   in vplus), are parked in SBUF at 32-aligned partitions, inverted with
   wide Ln+Exp pairs, and applied to ctxT right before the o-proj (ctx is
   staged unnormalized in bf16 - same relative precision).
 - MLP weights stream early: pool lifetimes are split (pq/pkv/px on the
   left pool stack, attention pool on the right) so the gate/up/down tiles
   reuse the K/V-phase SBUF ring addresses, and their DMAs sit last on the
   sync/gpsimd queues where ring-slot pacing can't head-of-line-block
   anything (baseline lost ~150us to MLP weights arriving at t=300us).
 - Attention packs (half, q-tile) into single 2-bank PSUM tiles: one exp
   per (key-tile, kv-pair), causal mask computed once per key-tile.
 - Down-proj is fused into the gate/up loop, accumulating into 4 per-bank
   PSUM tiles so the gated-update tail can drain per bank pair.
 - rmsnorm chunks and K/V projections are software-pipelined; the PE
   stream has no cross-engine waits once the first x-chunk lands.
"""

import sys

sys.path.insert(0, "/opt/trn_rl_repo")

import numpy as np
import ml_dtypes

import concourse.bass as bass
import concourse.tile as tile
from concourse import mybir
from concourse.bass_utils import run_bass_kernel_spmd
from concourse.vector_clock import ScopedClock, VectorClock

BF16 = mybir.dt.bfloat16
F32 = mybir.dt.float32
AF = mybir.ActivationFunctionType
OP = mybir.AluOpType

B, T, D = 4, 2048, 1024
H, KV, HD = 16, 8, 64
DFF = 4096
KSEL = 512

NQ = 256          # queries per core
ND = D // 128     # 8 d-tiles
NT = T // 128     # 16 key tiles
NKC = KV * HD // 128   # 4 k-output chunks (2 kv heads each)
NQC = H * HD // 128    # 8 q-output chunks (2 q heads each)
NFC = DFF // 128       # 32 ff chunks
NCORES = 8
NCH = 4           # 512-token chunks

# q-head layout: q-chunk tile 2c holds heads (4c, 4c+2) on partition halves
# (kv heads 2c / 2c+1), tile 2c+1 holds (4c+1, 4c+3). kv head of q-head h is h//2.
TILE_HEADS = []
for c in range(4):
    TILE_HEADS.append((4 * c, 4 * c + 2))
    TILE_HEADS.append((4 * c + 1, 4 * c + 3))
HEAD_PERM = np.array([h * HD + i for pair in TILE_HEADS for h in pair for i in range(HD)])


# ---------------------------------------------------------------------------
# walrus workarounds: this toolchain encodes at most ONE semaphore wait per
# instruction. Split the tile tail-drain into per-proc drains and move excess
# waits onto NoOps.
# ---------------------------------------------------------------------------

def _patched_drain_and_barrier(self, tick_clock, wait_clock):
    gc = tick_clock.global_clock
    n = len(gc)
    for i in range(n):
        t = gc[i]
        if t > 0:
            vec = [0] * n
            vec[i] = t
            d = self.nc.sync.drain()
            wait_clock.add_sem_waits(d.ins, ScopedClock({None: VectorClock(vec)}))
    self.nc.all_engine_barrier()
    popped = self.nc._tile_sem_poison_stack.pop()
    assert popped is self._sem_poison
    self.nc.clear_and_free_semaphores(list(self.sems.allocated().values()))
    self.nc.all_engine_barrier()


tile.TileContext._drain_and_barrier = _patched_drain_and_barrier

_MAX_WAITS = 1


def _split_excess_waits(nc):
    for f in nc.m.functions:
        for bb in f.blocks:
            new = []
            for inst in bb.instructions:
                si = inst.sync_info
                if si is not None and si.on_wait is not None and len(si.on_wait) > _MAX_WAITS:
                    waits = list(si.on_wait)
                    excess, keep = waits[:-_MAX_WAITS], waits[-_MAX_WAITS:]
                    k = 0
                    while excess:
                        chunk, excess = excess[:_MAX_WAITS], excess[_MAX_WAITS:]
                        new.append(mybir.InstNoOp(
                            name=f"{inst.name}_ws{k}",
                            engine=inst.engine,
                            sync_info=mybir.SyncInfo(on_wait=chunk, on_update=[])))
                        k += 1
                    inst.sync_info = mybir.SyncInfo(
                        on_wait=keep, on_update=list(si.on_update or []))
                new.append(inst)
            bb.instructions = new


def _bcast2(ap, n0, n1):
    """View a [128, w] slice as [128, n0, n1, w] with 0-stride bcast dims."""
    return bass.AP(tensor=ap.tensor, offset=ap.offset,
                   ap=[ap.ap[0], [0, n0], [0, n1]] + list(ap.ap[1:]))


# ---------------------------------------------------------------------------
# device program
# ---------------------------------------------------------------------------

def build_program(qlo, qhi, dbg=False):
    """qlo/qhi: per key-tile [NT] compile-time query ranges (uniform across cores).

    For key tile tt only queries [qlo[tt]:NQ) attend any of its keys; queries in
    [qlo[tt]:qhi[tt]) are partially masked, [qhi[tt]:NQ) fully valid.
    """
    nc = bass.Bass(trn_type="TRN2", target_bir_lowering=False, debug=False)

    def inp(name, shape, dt):
        return nc.dram_tensor(name, shape, dt, kind="ExternalInput").ap()

    hiddenT = inp("hiddenT", [128, NCH, ND, 512], BF16)  # chunk-major, contig
    selresT = inp("selresT", [128, ND, NQ], F32)
    qwT = inp("qwT", [128, ND, H * HD], BF16)
    kwT = inp("kwT", [128, ND, KV * HD], BF16)
    vwT = inp("vwT", [128, ND, KV * HD], BF16)
    owT = inp("owT", [128, NQC, D], BF16)
    gw = inp("gw", [NFC, 128, ND, 128], BF16)
    uw = inp("uw", [NFC, 128, ND, 128], BF16)
    dw = inp("dw", [NFC, 128, ND, 128], BF16)   # ft-major: [ft][128ff, ND, 128d]
    # packed constants: tiny mask pack first (posq | tvals), then the f32
    # pack (qb | kb | vb | gmul), then bf16 pack (rope_m | cos_q | sin_q)
    CM = NQ + NT
    CF = NQC + NKC + KV * HD + NQ
    CB = 128 + NQ + NQ
    constm = inp("constm", [128, CM], F32)
    constf = inp("constf", [128, CF], F32)
    constb = inp("constb", [128, CB], BF16)
    cos_k = inp("cos_k", [128, T], BF16)
    sin_k = inp("sin_k", [128, T], BF16)

    updT = nc.dram_tensor("updT", [ND, 128, NQ], F32, kind="ExternalOutput").ap()
    dbg_o = {}
    if dbg:
        for nm, shp, dt_ in [("d_normT", [NCH, 128, ND, 512], BF16),
                             ("d_kT", [NKC, 128, T], BF16),
                             ("d_vplus", [NT, 128, KV, HD + 2], BF16),
                             ("d_qrT", [NQC, 128, NQ], BF16),
                             ("d_nselT", [ND, 128, NQ], BF16),
                             ("d_ctxT", [NQC, 128, NQ], BF16),
                             ("d_hTt", [ND, 128, NQ], F32),
                             ("d_n2T", [ND, 128, NQ], BF16),
                             ("d_rsum", [128, 2, 2, 2, NQ], F32)]:
            dbg_o[nm] = nc.dram_tensor(nm, shp, dt_, kind="ExternalOutput").ap()

    live = [t_ for t_ in range(NT) if qlo[t_] < NQ]
    last_tt = max(live)
    mw = max(max(qhi[t_] - qlo[t_] for t_ in range(NT)), 1)
    mw = (mw + 15) // 16 * 16

    with tile.TileContext(nc, pool_alloc_mode="queue") as tc:
        with tc.tile_pool(name="pp", bufs=1) as pp, \
             tc.tile_pool(name="rowp", bufs=2) as rowp:

            # ---- persistent tiles -------------------------------------------
            selT = pp.tile([128, ND, NQ], F32, name="selT")
            hTt = pp.tile([128, ND, NQ], F32, name="hTt")
            n2T = pp.tile([128, ND, NQ], BF16, name="n2T")
            preT = pp.tile([128, ND, NQ], F32, name="preT")
            ones_bf = pp.tile([128, 128], BF16, name="ones_bf")
            nc.vector.memset(ones_bf, 1.0)

            c_m = pp.tile([128, CM], F32, name="c_m")
            c_f = pp.tile([128, CF], F32, name="c_f")
            c_b = pp.tile([128, CB], BF16, name="c_b")
            nc.scalar.dma_start(out=c_m, in_=constm)
            nc.scalar.dma_start(out=c_f, in_=constf)
            nc.scalar.dma_start(out=c_b, in_=constb)
            c_pos = c_m[:, 0:NQ]
            c_tv = c_m[:, NQ:NQ + NT]
            o_ = 0
            c_qb = c_f[:, o_:o_ + NQC]; o_ += NQC
            c_kb = c_f[:, o_:o_ + NKC]; o_ += NKC
            c_vb = c_f[:, o_:o_ + KV * HD]; o_ += KV * HD
            c_g = c_f[:, o_:o_ + NQ]; o_ += NQ
            c_rm = c_b[:, 0:128]
            c_cq = c_b[:, 128:128 + NQ]
            c_sq = c_b[:, 128 + NQ:128 + 2 * NQ]

            # attention-lifetime pool (exits after o-proj); lives on the
            # "right" pool stack so its mid-kernel release doesn't violate
            # LIFO order against the MLP weight pools on the left stack.
            pattn_cm = tc.tile_pool(name="pattn", bufs=1, side="right")
            pattn = pattn_cm.__enter__()
            kT = pattn.tile([128, NKC, T], BF16, name="kT")
            vplus = pattn.tile([128, NT, KV, HD + 2], BF16, name="vplus")
            qrT = pattn.tile([128, NQC, NQ], BF16, name="qrT")
            w_o = pattn.tile([128, NQC, D], BF16, name="w_o")
            ctxT = pattn.tile([128, NQC, NQ], BF16, name="ctxT")
            # rowsum parking: kc<3 -> (partition 32*kc, slot 0); kc=3 ->
            # (partition 64, slot 1). Matmul rhs partition base must be
            # 0/32/64, so partition 96 is unusable.
            rsum = pattn.tile([128, 2, 2, 2, NQ], F32, name="rsum")
            rcpr = pattn.tile([128, 2, 2, 2, NQ], BF16, name="rcpr")
            c_ck = pattn.tile([128, T], BF16, name="c_ck")
            c_sk = pattn.tile([128, T], BF16, name="c_sk")
            maskall = pattn.tile([128, NT, mw], BF16, name="maskall")

            # K/V + Q weight pools (freed before MLP weights stream in);
            # pq entered first so exits stay LIFO: px, pkv, then pq.
            pq_cm = tc.tile_pool(name="pq", bufs=1)
            pq = pq_cm.__enter__()
            w_q = pq.tile([128, ND, H * HD], BF16, name="w_q")
            nselT = pq.tile([128, ND, NQ], BF16, name="nselT")
            pkv_cm = tc.tile_pool(name="pkv", bufs=1)
            pkv = pkv_cm.__enter__()
            w_k = pkv.tile([128, ND, KV * HD], BF16, name="w_k")
            w_v = pkv.tile([128, ND, KV * HD], BF16, name="w_v")

            # gpsimd queue: weights + rope tables, one issue each
            nc.gpsimd.dma_start(out=w_k, in_=kwT)
            nc.gpsimd.dma_start(out=w_v, in_=vwT)
            nc.gpsimd.dma_start(out=c_ck, in_=cos_k)
            nc.gpsimd.dma_start(out=c_sk, in_=sin_k)
            nc.gpsimd.dma_start(out=w_q, in_=qwT)
            nc.gpsimd.dma_start(out=w_o, in_=owT)
            nc.scalar.dma_start(out=selT, in_=selresT)

            nc.vector.memset(vplus[:, :, :, 0:1], 1.0)
            nc.vector.memset(vplus[:, :, :, HD + 1:HD + 2], 1.0)

            # =================================================================
            # Phase 1+2: per-chunk rmsnorm (b1) interleaved with K/V
            # projections (kproj/vproj) so the PE never waits on a whole-pass
            # barrier; the selected-row norm rides along near the end.
            # =================================================================
            px_cm = tc.tile_pool(name="px", bufs=1)
            px = px_cm.__enter__()
            xch = px.tile([128, NCH, ND, 512], BF16, name="xch")

            with tc.tile_pool(name="p12", bufs=3) as p12, \
                 tc.tile_pool(name="ps12", bufs=1, space="PSUM") as ps12:
                for ch in range(NCH):
                    eng = nc.sync if ch % 2 == 0 else nc.scalar
                    eng.dma_start(out=xch[:, ch], in_=hiddenT[:, ch])

                def b1(ch):
                    bs = ps12.tile([128, 512], F32, name="bs", tag="bs", bufs=2)
                    for dt in range(ND):
                        sq = p12.tile([128, 512], BF16, name="sq")
                        nc.vector.tensor_mul(
                            out=sq, in0=xch[:, ch, dt, :], in1=xch[:, ch, dt, :])
                        nc.tensor.matmul(bs, lhsT=ones_bf, rhs=sq,
                                         start=(dt == 0), stop=(dt == ND - 1))
                    lnt = p12.tile([128, 512], F32, name="lnt")
                    nc.scalar.activation(out=lnt, in_=bs, func=AF.Ln,
                                         scale=1.0 / D)
                    rstd = p12.tile([128, 512], BF16, name="rstd")
                    nc.scalar.activation(out=rstd, in_=lnt, func=AF.Exp,
                                         scale=-0.5)
                    rstd_b = bass.AP(tensor=rstd.tensor, offset=rstd.offset,
                                     ap=[rstd.ap[0], [0, ND], rstd.ap[1]])
                    nc.vector.tensor_mul(out=xch[:, ch], in0=xch[:, ch],
                                         in1=rstd_b)
                    if dbg:
                        nc.sync.dma_start(out=dbg_o["d_normT"][ch],
                                          in_=xch[:, ch])

                def kproj(ch):
                    cs = slice(ch * 512, (ch + 1) * 512)
                    for kc in range(NKC):
                        kps = ps12.tile([128, 512], F32, name="kps", tag="kps",
                                        bufs=2)
                        for dt in range(ND):
                            nc.tensor.matmul(
                                kps, lhsT=w_k[:, dt, kc * 128:(kc + 1) * 128],
                                rhs=xch[:, ch, dt, :],
                                start=(dt == 0), stop=(dt == ND - 1))
                        kraw = p12.tile([128, 512], BF16, name="kraw")
                        nc.vector.tensor_scalar(
                            out=kraw, in0=kps, scalar1=c_kb[:, kc:kc + 1],
                            scalar2=None, op0=OP.add)
                        rot = ps12.tile([128, 512], F32, name="rot", tag="rot",
                                        bufs=2)
                        nc.tensor.matmul(rot, lhsT=c_rm, rhs=kraw,
                                         start=True, stop=True)
                        dst = kT[:, kc, cs]
                        tmp = p12.tile([128, 512], BF16, name="tmp")
                        nc.vector.tensor_mul(out=tmp, in0=rot, in1=c_sk[:, cs])
                        nc.vector.tensor_mul(out=dst, in0=kraw, in1=c_ck[:, cs])
                        nc.vector.tensor_add(out=dst, in0=dst, in1=tmp)

                def vproj(ch):
                    for tt in range(ch * 4, ch * 4 + 4):
                        vps = ps12.tile([128, 512], F32, name="vps", tag="vps",
                                        bufs=2)
                        for dt in range(ND):
                            nc.tensor.matmul(
                                vps, lhsT=xch[:, ch, dt, tt % 4 * 128:(tt % 4 + 1) * 128],
                                rhs=w_v[:, dt, :],
                                start=(dt == 0), stop=(dt == ND - 1))
                        nc.vector.tensor_add(
                            out=vplus[:, tt, :, 1:HD + 1],
                            in0=vps.rearrange("p (h d) -> p h d", h=KV),
                            in1=c_vb.rearrange("p (h d) -> p h d", h=KV))

                def selnorm():
                    bs3 = ps12.tile([128, NQ], F32, name="bs3", tag="bs",
                                    bufs=2)
                    for dt in range(ND):
                        sq3 = p12.tile([128, NQ], BF16, name="sq3")
                        nc.vector.tensor_mul(out=sq3, in0=selT[:, dt, :],
                                             in1=selT[:, dt, :])
                        nc.tensor.matmul(bs3, lhsT=ones_bf, rhs=sq3,
                                         start=(dt == 0), stop=(dt == ND - 1))
                    ln3 = p12.tile([128, NQ], F32, name="ln3")
                    nc.scalar.activation(out=ln3, in_=bs3, func=AF.Ln,
                                         scale=1.0 / D)
                    rstd3 = p12.tile([128, NQ], BF16, name="rstd3")
                    nc.scalar.activation(out=rstd3, in_=ln3, func=AF.Exp,
                                         scale=-0.5)
                    rstd3_b = bass.AP(tensor=rstd3.tensor, offset=rstd3.offset,
                                      ap=[rstd3.ap[0], [0, ND], rstd3.ap[1]])
                    nc.vector.tensor_mul(out=nselT[:, :, :], in0=selT[:, :, :],
                                         in1=rstd3_b)

                b1(0)
                b1(1)
                kproj(0)
                vproj(0)
                b1(2)
                kproj(1)
                vproj(1)
                b1(3)
                kproj(2)
                vproj(2)
                selnorm()
                kproj(3)
                vproj(3)

            px_cm.__exit__(None, None, None)
            pkv_cm.__exit__(None, None, None)

            # masks once per key tile: mask[p, q] = (pos_q >= tt*128 + p)
            for tt in range(NT):
                lo, hi = qlo[tt], qhi[tt]
                if lo >= NQ or hi <= lo:
                    continue
                nc.vector.tensor_scalar(
                    out=maskall[:, tt, 0:hi - lo], in0=c_pos[:, lo:hi],
                    scalar1=c_tv[:, tt:tt + 1], scalar2=None, op0=OP.is_ge)

            # =================================================================
            # Phase 3: Q proj + rope (selected-row norm already done)
            # =================================================================
            with tc.tile_pool(name="p3", bufs=3) as p3, \
                 tc.tile_pool(name="ps3", bufs=1, space="PSUM") as ps3:
                for qc in range(NQC):
                    qps = ps3.tile([128, NQ], F32, name="qps", tag="qps", bufs=2)
                    for dt in range(ND):
                        nc.tensor.matmul(
                            qps, lhsT=w_q[:, dt, qc * 128:(qc + 1) * 128],
                            rhs=nselT[:, dt, :],
                            start=(dt == 0), stop=(dt == ND - 1))
                    qraw = p3.tile([128, NQ], BF16, name="qraw")
                    nc.vector.tensor_scalar(
                        out=qraw, in0=qps, scalar1=c_qb[:, qc:qc + 1],
                        scalar2=None, op0=OP.add)
                    rotq = ps3.tile([128, NQ], F32, name="rotq", tag="rotq",
                                    bufs=2)
                    nc.tensor.matmul(rotq, lhsT=c_rm, rhs=qraw,
                                     start=True, stop=True)
                    dst = qrT[:, qc, :]
                    tmpq = p3.tile([128, NQ], BF16, name="tmpq")
                    nc.vector.tensor_mul(out=tmpq, in0=rotq, in1=c_sq)
                    nc.vector.tensor_mul(out=dst, in0=qraw, in1=c_cq)
                    nc.vector.tensor_add(out=dst, in0=dst, in1=tmpq)

            pq_cm.__exit__(None, None, None)

            if dbg:
                for kc in range(NKC):
                    nc.scalar.dma_start(out=dbg_o["d_kT"][kc], in_=kT[:, kc, :])
                for tt in range(NT):
                    nc.scalar.dma_start(out=dbg_o["d_vplus"][tt], in_=vplus[:, tt])
                for qc in range(NQC):
                    nc.scalar.dma_start(out=dbg_o["d_qrT"][qc], in_=qrT[:, qc, :])
                for dt in range(ND):
                    nc.scalar.dma_start(out=dbg_o["d_nselT"][dt], in_=nselT[:, dt, :])

            # =================================================================
            # MLP weight streams: pools enter AFTER the K/V pools exit so their
            # ring addresses reuse that SBUF; DMA issues sit last on their
            # queues (sync: gate+up; gpsimd: first 8 down tiles).
            # =================================================================
            p7g_cm = tc.tile_pool(name="p7g", bufs=1)
            p7g = p7g_cm.__enter__()
            p7u_cm = tc.tile_pool(name="p7u", bufs=1)
            p7u = p7u_cm.__enter__()
            p8w_cm = tc.tile_pool(name="p8w", bufs=1)
            p8w = p8w_cm.__enter__()
            wg_tiles, wu_tiles, wd_tiles = [], [], []
            for fc in range(NFC):
                wg_t = p7g.tile([128, ND, 128], BF16, name=f"wg{fc}", tag="wg",
                                bufs=12)
                nc.sync.dma_start(out=wg_t, in_=gw[fc])
                wg_tiles.append(wg_t)
                wu_t = p7u.tile([128, ND, 128], BF16, name=f"wu{fc}", tag="wu",
                                bufs=12)
                nc.sync.dma_start(out=wu_t, in_=uw[fc])
                wu_tiles.append(wu_t)
            for ft in range(8):
                wd_t = p8w.tile([128, ND, 128], BF16, name=f"wd{ft}", tag="wd",
                                bufs=10)
                nc.gpsimd.dma_start(out=wd_t, in_=dw[ft])
                wd_tiles.append(wd_t)

            # =================================================================
            # Phase 4: attention. Per kv-pair kc: per key tile: scores for both
            # halves x both q-tiles into one 2-bank PSUM tile, one exp, masked
            # mul, ctx accumulate (rowsums ride along via vplus ones column).
            # =================================================================
            with tc.tile_pool(name="p4", bufs=1) as p4, \
                 tc.tile_pool(name="ps4", bufs=1, space="PSUM") as ps4:
                cps_all = {}

                def attn_tloop(kc):
                    cps = ps4.tile([128, 2, 2, NQ], F32, name=f"cps{kc}",
                                   tag=f"cps{kc % 2}", bufs=1)
                    cps_all[kc] = cps
                    for tt in range(NT):
                        lo, hi = qlo[tt], qhi[tt]
                        if lo >= NQ:
                            continue
                        sp = ps4.tile([128, 2, 2, NQ], F32, name="sp", tag="sp",
                                      bufs=2)
                        for half in range(2):
                            hs_ = slice(half * 64, (half + 1) * 64)
                            nc.tensor.matmul(
                                sp[:, half, :, lo:NQ],
                                lhsT=kT[hs_, kc, tt * 128:(tt + 1) * 128],
                                rhs=qrT[hs_, 2 * kc:2 * kc + 2, lo:NQ],
                                start=True, stop=True)
                        pt = p4.tile([128, 2, 2, NQ], BF16, name="pt", tag="pt",
                                     bufs=6)
                        nc.scalar.activation(
                            out=pt[:, :, :, lo:NQ], in_=sp[:, :, :, lo:NQ],
                            func=AF.Exp)
                        if hi > lo:
                            m = maskall[:, tt, 0:hi - lo]
                            nc.vector.tensor_mul(
                                out=pt[:, :, :, lo:hi], in0=pt[:, :, :, lo:hi],
                                in1=_bcast2(m, 2, 2))
                        for half in range(2):
                            nc.tensor.matmul(
                                cps[0:HD + 1, half, :, lo:NQ],
                                lhsT=vplus[:, tt, 2 * kc + half, 1:HD + 2],
                                rhs=pt[:, half, :, lo:NQ],
                                start=(tt == live[0]), stop=(tt == last_tt))

                def attn_evict(kc):
                    cps = cps_all[kc]
                    # upper-half relocation first (its SBUF->SBUF DMA is the
                    # long pole into the o-proj); rowsums parked at 32-aligned
                    # partition; ctx copied out unnormalized.
                    pb, sl = (32 * kc, 0) if kc < 3 else (64, 1)
                    stage = p4.tile([64, 2, NQ], BF16, name="stage", tag="stage",
                                    bufs=2)
                    nc.vector.tensor_copy(out=stage, in_=cps[0:HD, 1, :, :])
                    nc.gpsimd.dma_start(out=ctxT[64:128, 2 * kc:2 * kc + 2, :],
                                        in_=stage)
                    nc.vector.tensor_copy(out=rsum[pb:pb + 1, sl],
                                          in_=cps[HD:HD + 1, :, :, :])
                    nc.vector.tensor_copy(out=ctxT[0:64, 2 * kc:2 * kc + 2, :],
                                          in_=cps[0:HD, 0, :, :])

                lnr = p4.tile([128, 2, 2, 2, NQ], F32, name="lnr", tag="lnr",
                              bufs=1)

                def ctx_norm(kcs, sl):
                    # 1/rowsum via Ln+Exp (wide; junk lanes ok), broadcast to
                    # both partition halves via one-row matmuls, scale ctxT.
                    nc.scalar.activation(out=lnr[:, sl], in_=rsum[:, sl],
                                         func=AF.Ln)
                    nc.scalar.activation(out=rcpr[:, sl], in_=lnr[:, sl],
                                         func=AF.Exp, scale=-1.0)
                    for kc in kcs:
                        pb = 32 * kc if sl == 0 else 64
                        rfac = ps4.tile([128, 2, NQ], F32, name="rfac",
                                        tag="sp", bufs=2)
                        ones_row = ones_bf[pb:pb + 1, 0:64]
                        nc.tensor.matmul(rfac[0:64], lhsT=ones_row,
                                         rhs=rcpr[pb:pb + 1, sl, 0, :, :],
                                         start=True, stop=True)
                        nc.tensor.matmul(rfac[64:128], lhsT=ones_row,
                                         rhs=rcpr[pb:pb + 1, sl, 1, :, :],
                                         start=True, stop=True)
                        nc.vector.tensor_mul(out=ctxT[:, 2 * kc:2 * kc + 2, :],
                                             in0=ctxT[:, 2 * kc:2 * kc + 2, :],
                                             in1=rfac)

                attn_tloop(0)
                attn_tloop(1)
                attn_evict(0)
                attn_tloop(2)
                attn_evict(1)
                attn_tloop(3)
                attn_evict(2)
                ctx_norm([0, 1, 2], 0)
                attn_evict(3)
                ctx_norm([3], 1)

            if dbg:
                nc.scalar.dma_start(out=dbg_o["d_rsum"], in_=rsum)

            # =================================================================
            # Phase 5: o-proj + residual -> hTt
            # =================================================================
            with tc.tile_pool(name="p5", bufs=2) as p5, \
                 tc.tile_pool(name="ps5", bufs=1, space="PSUM") as ps5:
                if dbg:
                    for qc in range(NQC):
                        nc.scalar.dma_start(out=dbg_o["d_ctxT"][qc],
                                            in_=ctxT[:, qc, :])

                # o-proj; the norm2 squares ride the vector queue per dc so
                # only the 8 batched sum-matmuls + Ln/Exp remain at the end
                sq6s = []
                for dc in range(ND):
                    ops_ = ps5.tile([128, NQ], F32, name="ops_", tag="ops",
                                    bufs=2)
                    for hc in range(NQC):
                        nc.tensor.matmul(
                            ops_, lhsT=w_o[:, hc, dc * 128:(dc + 1) * 128],
                            rhs=ctxT[:, hc, :], start=(hc == 0),
                            stop=(hc == NQC - 1))
                    nc.vector.tensor_add(out=hTt[:, dc, :], in0=ops_,
                                         in1=selT[:, dc, :])
                    sq6 = p5.tile([128, NQ], BF16, name="sq6", tag="sq6",
                                  bufs=8)
                    nc.vector.tensor_mul(out=sq6, in0=hTt[:, dc, :],
                                         in1=hTt[:, dc, :])
                    sq6s.append(sq6)
                bs6 = ps5.tile([128, NQ], F32, name="bs6", tag="bs6", bufs=1)
                for dt in range(ND):
                    nc.tensor.matmul(bs6, lhsT=ones_bf, rhs=sq6s[dt],
                                     start=(dt == 0), stop=(dt == ND - 1))
                ln6 = p5.tile([128, NQ], F32, name="ln6", tag="ln6", bufs=1)
                nc.scalar.activation(out=ln6, in_=bs6, func=AF.Ln, scale=1.0 / D)
                rstd6 = p5.tile([128, NQ], BF16, name="rstd6", tag="rstd6",
                                bufs=1)
                nc.scalar.activation(out=rstd6, in_=ln6, func=AF.Exp, scale=-0.5)
                for dt in range(ND):
                    nc.vector.tensor_mul(out=n2T[:, dt, :], in0=hTt[:, dt, :],
                                         in1=rstd6)

            pattn_cm.__exit__(None, None, None)
            if dbg:
                for dt in range(ND):
                    nc.scalar.dma_start(out=dbg_o["d_hTt"][dt], in_=hTt[:, dt, :])
                for dt in range(ND):
                    nc.scalar.dma_start(out=dbg_o["d_n2T"][dt], in_=n2T[:, dt, :])

            # =================================================================
            # Phase 7: fused MLP: gate/up + silu-mul, down accumulated into an
            # 8-region PSUM as each act chunk is produced; gated update + out.
            # =================================================================
            with tc.tile_pool(name="p7", bufs=4) as p7, \
                 tc.tile_pool(name="ps7", bufs=1, space="PSUM") as ps7:
                # one accumulator tile per PSUM bank (dc pair) so the gated
                # update for a dc pair can start as soon as its bank stops
                mps = [ps7.tile([128, 2, NQ], F32, name=f"mps{i}",
                                tag=f"mps{i}", bufs=1) for i in range(ND // 2)]
                act_tiles = [None] * NFC

                def gate_up(fc):
                    gu = ps7.tile([128, 2, NQ], F32, name="gu", tag="gu", bufs=2)
                    for dt in range(ND):
                        nc.tensor.matmul(gu[:, 0, :], lhsT=wg_tiles[fc][:, dt, :],
                                         rhs=n2T[:, dt, :],
                                         start=(dt == 0), stop=False,
                                         skip_group_check=True)
                    for dt in range(ND):
                        nc.tensor.matmul(gu[:, 1, :], lhsT=wu_tiles[fc][:, dt, :],
                                         rhs=n2T[:, dt, :],
                                         start=False, stop=(dt == ND - 1),
                                         skip_group_check=True)
                    sg = p7.tile([128, NQ], BF16, name="sg", tag="sg")
                    nc.scalar.activation(out=sg, in_=gu[:, 0, :], func=AF.Silu)
                    act = p7.tile([128, NQ], BF16, name="act", tag="act")
                    nc.vector.tensor_mul(out=act, in0=gu[:, 1, :], in1=sg)
                    act_tiles[fc] = act

                def down(ft):
                    for dc in range(ND):
                        nc.tensor.matmul(
                            mps[dc // 2][:, dc % 2, :],
                            lhsT=wd_tiles[ft][:, dc, :],
                            rhs=act_tiles[ft],
                            start=(ft == 0 and dc % 2 == 0),
                            stop=(ft == NFC - 1),
                            skip_group_check=True)

                # pre = selres + g*(h - selres): computed on the DVE during
                # the MLP window (vector is mostly idle there), leaving a
                # 2-op tail per dc: out = pre + g*mlp.
                def pre_op(k):
                    dc = k % ND
                    if k < ND:
                        nc.vector.tensor_sub(out=preT[:, dc, :],
                                             in0=hTt[:, dc, :],
                                             in1=selT[:, dc, :])
                    elif k < 2 * ND:
                        nc.vector.tensor_mul(out=preT[:, dc, :],
                                             in0=preT[:, dc, :], in1=c_g)
                    else:
                        nc.vector.tensor_add(out=preT[:, dc, :],
                                             in0=preT[:, dc, :],
                                             in1=selT[:, dc, :])

                gate_up(0)
                for fc in range(1, NFC):
                    # stream remaining down-weight tiles on gpsimd, paced
                    if fc + 7 < NFC:
                        ft2 = fc + 7
                        wd_t = p8w.tile([128, ND, 128], BF16, name=f"wd{ft2}",
                                        tag="wd", bufs=10)
                        nc.gpsimd.dma_start(out=wd_t, in_=dw[ft2])
                        wd_tiles.append(wd_t)
                    gate_up(fc)
                    down(fc - 1)
                    if fc <= 3 * ND:
                        pre_op(fc - 1)
                down(NFC - 1)

                # updated = pre + g*mlp
                for dc in range(ND):
                    f1 = p7.tile([128, NQ], F32, name="f1", tag="f1", bufs=8)
                    nc.vector.tensor_mul(out=f1, in0=mps[dc // 2][:, dc % 2, :],
                                         in1=c_g)
                    nc.vector.tensor_add(out=f1, in0=f1, in1=preT[:, dc, :])
                    nc.scalar.dma_start(out=updT[dc], in_=f1)

            p8w_cm.__exit__(None, None, None)
            p7u_cm.__exit__(None, None, None)
            p7g_cm.__exit__(None, None, None)

    _split_excess_waits(nc)
    return nc


# ---------------------------------------------------------------------------
# host side
# ---------------------------------------------------------------------------

def _bf16(x):
    return np.asarray(x, dtype=np.float32).astype(ml_dtypes.bfloat16)


def _rope_matrix():
    """R[k, p] = sign(p) * 1[k == swap(p)]; (R.T @ x)[p] = sign(p)*x[swap(p)].

    rot(x)[p%64 < 32] = -x[p+32], else +x[p-32]  (two stacked 64-dim heads).
    """
    R = np.zeros((128, 128), np.float32)
    for p in range(128):
        base = (p // 64) * 64
        off = p % 64
        if off < 32:
            R[base + off + 32, p] = -1.0
        else:
            R[base + off - 32, p] = 1.0
    return R


def _install_ntff_hook():
    """Shim antenv.axon_hooks (absent in this image) so trace=True works."""
    import types
    try:
        import antenv.axon_hooks  # noqa: F401
        return
    except ImportError:
        pass
    try:
        from trn_agent_boot.trn_boot import _ntff_profile_via_ctypes
        hook = _ntff_profile_via_ctypes("/opt/axon/libaxon_pjrt.so")
    except Exception:
        hook = None
    mod = types.ModuleType("antenv.axon_hooks")
    mod._hook = hook
    mod.set_axon_ntff_profile_hook = lambda h: setattr(mod, "_hook", h)
    mod.get_axon_ntff_profile_hook = lambda: mod._hook
    sys.modules["antenv.axon_hooks"] = mod


def kernel(hidden_states, token_indices, batch_indices, gating_scores, cos, sin,
           ln1_w, ln2_w, q_w, q_b, k_w, k_b, v_w, v_b, o_w, gate_w, up_w, down_w,
           _profile=False, _dbg=False):
    hidden_states = np.asarray(hidden_states, dtype=np.float32)
    token_indices = np.asarray(token_indices).astype(np.int64)
    gating_scores = np.asarray(gating_scores, dtype=np.float32)
    cos = np.asarray(cos, dtype=np.float32)
    sin = np.asarray(sin, dtype=np.float32)
    ln1_w = np.asarray(ln1_w, dtype=np.float32)
    ln2_w = np.asarray(ln2_w, dtype=np.float32)

    topk = token_indices.reshape(B, KSEL)
    gsc = gating_scores.reshape(B, KSEL)

    core_pos = []
    for c in range(NCORES):
        b = c // 2
        core_pos.append(np.asarray(topk[b, c % 2::2], dtype=np.int64))

    qlo = [min(int(np.searchsorted(core_pos[c], tt * 128)) for c in range(NCORES))
           for tt in range(NT)]
    qhi = [max(int(np.searchsorted(core_pos[c], tt * 128 + 126, side="right"))
               for c in range(NCORES))
           for tt in range(NT)]

    nc = build_program(qlo, qhi, dbg=_dbg)

    # ---- weights (shared across cores) ----
    q_w_eff = (np.asarray(q_w, np.float32) * ln1_w[None, :]) / 8.0
    k_w_eff = np.asarray(k_w, np.float32) * ln1_w[None, :]
    v_w_eff = np.asarray(v_w, np.float32) * ln1_w[None, :]
    g_w_eff = np.asarray(gate_w, np.float32) * ln2_w[None, :]
    u_w_eff = np.asarray(up_w, np.float32) * ln2_w[None, :]
    q_b_eff = (np.asarray(q_b, np.float32) / 8.0)[HEAD_PERM]

    def pm(x):     # [ND, 128, n] -> [128, ND, n] partition-major
        return np.ascontiguousarray(x.transpose(1, 0, 2))

    qwT = _bf16(pm(q_w_eff.T[:, HEAD_PERM].reshape(ND, 128, H * HD)))
    kwT = _bf16(pm(k_w_eff.T.reshape(ND, 128, KV * HD)))
    vwT = _bf16(pm(v_w_eff.T.reshape(ND, 128, KV * HD)))
    owT = _bf16(pm(np.asarray(o_w, np.float32).T[HEAD_PERM, :].reshape(NQC, 128, D)
                   ))
    gwa = _bf16(np.ascontiguousarray(
        g_w_eff.reshape(NFC, 128, ND, 128).transpose(0, 3, 2, 1)))
    uwa = _bf16(np.ascontiguousarray(
        u_w_eff.reshape(NFC, 128, ND, 128).transpose(0, 3, 2, 1)))
    dwa = _bf16(np.ascontiguousarray(
        np.asarray(down_w, np.float32).reshape(ND, 128, NFC, 128)
        .transpose(2, 3, 0, 1)))

    qb_a = np.ascontiguousarray(q_b_eff.reshape(NQC, 128).T).astype(np.float32)
    kb_a = np.ascontiguousarray(np.asarray(k_b, np.float32).reshape(NKC, 128).T)
    vb_a = np.broadcast_to(np.asarray(v_b, np.float32)[None, :], (128, KV * HD)).copy()
    tvals = (np.arange(NT)[None, :] * 128 + np.arange(128)[:, None]).astype(np.float32)

    shared = dict(qwT=qwT, kwT=kwT, vwT=vwT, owT=owT, gw=gwa, uw=uwa, dw=dwa)

    def stack2(mat):        # [n, 64] -> [128, n] (head-pair stacked transpose)
        mT = mat.T.astype(np.float32)
        return np.concatenate([mT, mT], axis=0)

    in_maps = []
    for c in range(NCORES):
        b = c // 2
        pos = core_pos[c]
        im = dict(shared)
        posq = np.broadcast_to(pos.astype(np.float32)[None, :], (128, NQ))
        gmul = np.broadcast_to(gsc[b, c % 2::2].astype(np.float32)[None, :],
                               (128, NQ))
        constm = np.concatenate([posq, tvals], axis=1).astype(np.float32)
        constf = np.concatenate([qb_a, kb_a, vb_a, gmul],
                                axis=1).astype(np.float32)
        constb = np.concatenate(
            [_rope_matrix(), stack2(cos[b][pos]), stack2(sin[b][pos])],
            axis=1).astype(np.float32)
        im.update(
            hiddenT=_bf16(np.ascontiguousarray(
                hidden_states[b].T.reshape(ND, 128, NCH, 512)
                .transpose(1, 2, 0, 3))),
            selresT=np.ascontiguousarray(
                hidden_states[b][pos].T.reshape(ND, 128, NQ)
                .transpose(1, 0, 2)).astype(np.float32),
            constm=np.ascontiguousarray(constm),
            constf=np.ascontiguousarray(constf),
            constb=_bf16(constb),
            cos_k=_bf16(stack2(cos[b])),
            sin_k=_bf16(stack2(sin[b])),
        )
        in_maps.append(im)

    if _profile:
        _install_ntff_hook()
    res = run_bass_kernel_spmd(nc, in_maps, core_ids=list(range(NCORES)),
                               trace=_profile)

    out = hidden_states.copy()
    for c in range(NCORES):
        b = c // 2
        upd = res.results[c]["updT"].reshape(D, NQ).T
        out[b, core_pos[c], :] = upd
    if _profile or _dbg:
        return out, res
    return out
